# revision 5
# baseline (speedup 1.0000x reference)
"""Causal single-head attention (B=8, S=2048, D=1024) on 8 TRN2 NeuronCores.

Sharding: data-parallel over batch — core b computes batch element b entirely.

Per-core pipeline (all matmuls bf16 with fp32 PSUM accumulation):
  1. inp [S,D] f32 -> cast bf16 -> DRAM scratch -> DMA-xbar transpose -> inpT [D,S]
  2. W* [e,d] f32 -> cast bf16 -> DRAM scratch -> transpose -> W*T [d,e]
  3. KT[e,s] = W_kT.T @ inpT (+bk on eviction), V[s,e] = inpT.T @ WvT (+bv)
  4. per q-block j (512 wide): QT_j = WqT.T @ inpT[:, j]  (+bq)
     ST[k,q] blocks = KT.T @ QT_j, causal-masked (additive -1e30 tiles,
     only diagonal-crossing blocks; fully-masked blocks skipped entirely),
     P = exp(scale*ST) -> bf16  (no max subtraction: |scale*ST| <~ 10)
     ctx[q,e] = sum_k P[k,q].T @ V[k,e]; rowsum via ones-column matmul;
     out = ctx * (1/rowsum) on eviction.
"""

import math

import ml_dtypes
import numpy as np

import concourse.bass as bass
import concourse.mybir as mybir
from concourse.bass_utils import run_bass_kernel_spmd
from concourse.tile import TileContext

F32 = mybir.dt.float32
BF16 = mybir.dt.bfloat16

B, S, D = 8, 2048, 1024
P = 128                # partitions
NS = S // P            # 16 s-chunks of 128
ND = D // P            # 8 d-chunks of 128
NE = D // P            # 8 e-chunks of 128
QB = 512               # q-block width (PSUM bank = 512 f32)
NQB = S // QB          # 4 q-blocks
NKC = S // P           # 16 k-chunks of 128
MASKVAL = -1.0e30
SCALE = float(np.float32(1.0) / np.sqrt(np.float32(S)))

_TRACE = False
LAST_RESULTS = None


def _build_nc():
    nc = bass.Bass()
    inp = nc.dram_tensor("inp", [S, D], F32, kind="ExternalInput")
    wq = nc.dram_tensor("wq", [D, D], F32, kind="ExternalInput")
    wk = nc.dram_tensor("wk", [D, D], F32, kind="ExternalInput")
    wv = nc.dram_tensor("wv", [D, D], F32, kind="ExternalInput")
    bq = nc.dram_tensor("bq", [D], F32, kind="ExternalInput")
    bk = nc.dram_tensor("bk", [D], F32, kind="ExternalInput")
    bv = nc.dram_tensor("bv", [D], F32, kind="ExternalInput")
    # 4 diagonal-block mask patterns, [k_rel(128), q_rel(512)], 0 or -1e30
    masks = nc.dram_tensor("masks", [4, P, QB], BF16, kind="ExternalInput")
    out = nc.dram_tensor("out", [S, D], F32, kind="ExternalOutput")

    with TileContext(nc) as tc:
        with (
            tc.tile_pool(name="dram", bufs=1, space="DRAM") as dram_pool,
            tc.tile_pool(name="const", bufs=1) as const_pool,
            tc.tile_pool(name="stage", bufs=2) as stage_pool,
            tc.tile_pool(name="wt", bufs=2) as wt_pool,
            tc.tile_pool(name="inpT", bufs=1) as inpT_pool,
            tc.tile_pool(name="kt", bufs=1) as kt_pool,
            tc.tile_pool(name="v", bufs=1) as v_pool,
            tc.tile_pool(name="qt", bufs=2) as qt_pool,
            tc.tile_pool(name="p", bufs=NKC) as p_pool,
            tc.tile_pool(name="outp", bufs=2) as out_pool,
            tc.tile_pool(name="recip", bufs=2) as recip_pool,
            tc.tile_pool(name="ps_sc", bufs=4, space="PSUM") as ps_sc,
            tc.tile_pool(name="ps_ctx", bufs=3, space="PSUM") as ps_ctx,
            tc.tile_pool(name="ps_sum", bufs=1, space="PSUM") as ps_sum,
        ):
            # ---- constants ----
            ones = const_pool.tile([P, 1], BF16, tag="ones")
            nc.vector.memset(ones[:], 1.0)

            bias_sb = {}
            for name, t in (("bq", bq), ("bk", bk)):
                bt = const_pool.tile([P, NE], F32, tag=f"bias_{name}", name=f"bias_{name}")
                nc.sync.dma_start(out=bt[:], in_=t.rearrange("(c p) -> p c", p=P))
                bias_sb[name] = bt

            mask_sb = const_pool.tile([P, 4 * QB], BF16, tag="masks")
            for m in range(4):
                nc.sync.dma_start(out=mask_sb[:, m * QB:(m + 1) * QB],
                                  in_=masks[m])

            # broadcast bv to all 128 partitions via ones-outer-product matmul
            ones_row = const_pool.tile([1, P], BF16, tag="ones_row")
            nc.vector.memset(ones_row[:], 1.0)
            bv_row = stage_pool.tile([1, D], F32, tag="st_f32")
            nc.sync.dma_start(out=bv_row[:], in_=bv[None, :])
            bv_row_bf = stage_pool.tile([1, D], BF16, tag="st_bf")
            nc.vector.tensor_copy(bv_row_bf[:], bv_row[:])
            bv_bcast = const_pool.tile([P, D], BF16, tag="bv_bcast")
            for eh in range(2):
                ps = ps_sc.tile([P, QB], F32, tag="sc", name="bvb_ps")
                nc.tensor.matmul(ps[:], lhsT=ones_row[:],
                                 rhs=bv_row_bf[:, eh * QB:(eh + 1) * QB],
                                 start=True, stop=True)
                nc.vector.tensor_copy(bv_bcast[:, eh * QB:(eh + 1) * QB], ps[:])

            # ---- input: load, cast to bf16, roundtrip through DRAM transposed ----
            inp_bf_dram = dram_pool.tile([S, D], BF16, tag="inp_bf_dram")
            for si in range(NS):
                st = stage_pool.tile([P, D], F32, tag="st_f32")
                nc.sync.dma_start(out=st[:], in_=inp[si * P:(si + 1) * P, :])
                cb = stage_pool.tile([P, D], BF16, tag="st_bf")
                nc.vector.tensor_copy(cb[:], st[:])
                nc.sync.dma_start(out=inp_bf_dram[si * P:(si + 1) * P, :], in_=cb[:])
            inpT = []
            for dc in range(ND):
                t = inpT_pool.tile([P, S], BF16, tag=f"inpT{dc}", name=f"inpT{dc}")
                nc.sync.dma_start_transpose(
                    out=t[:], in_=inp_bf_dram[:, dc * P:(dc + 1) * P])
                inpT.append(t)

            # ---- weight prep helper: W [e,d] -> list of [128d, 1024e] bf16 ----
            def prep_weight(w, wname):
                w_bf_dram = dram_pool.tile([D, D], BF16, tag=f"{wname}_bf")
                for ec in range(NE):
                    st = stage_pool.tile([P, D], F32, tag="st_f32")
                    nc.sync.dma_start(out=st[:], in_=w[ec * P:(ec + 1) * P, :])
                    cb = stage_pool.tile([P, D], BF16, tag="st_bf")
                    nc.scalar.activation(cb[:], st[:],
                                         mybir.ActivationFunctionType.Copy)
                    nc.sync.dma_start(out=w_bf_dram[ec * P:(ec + 1) * P, :],
                                      in_=cb[:])
                wT = []
                for dc in range(ND):
                    t = wt_pool.tile([P, D], BF16, tag=f"wt{dc}", name=f"{wname}T{dc}")
                    nc.sync.dma_start_transpose(
                        out=t[:], in_=w_bf_dram[:, dc * P:(dc + 1) * P])
                    wT.append(t)
                return wT

            # ---- K projection: KT[e,s] tiles [128, 2048] ----
            wkT = prep_weight(wk, "wk")
            KT = [kt_pool.tile([P, S], BF16, tag=f"kt{ec}", name=f"kt{ec}")
                  for ec in range(NE)]
            for ec in range(NE):
                for sb in range(NQB):
                    ps = ps_sc.tile([P, QB], F32, tag="sc")
                    for dc in range(ND):
                        nc.tensor.matmul(
                            ps[:],
                            lhsT=wkT[dc][:, ec * P:(ec + 1) * P],
                            rhs=inpT[dc][:, sb * QB:(sb + 1) * QB],
                            start=(dc == 0), stop=(dc == ND - 1))
                    nc.scalar.activation(
                        KT[ec][:, sb * QB:(sb + 1) * QB], ps[:],
                        mybir.ActivationFunctionType.Identity,
                        bias=bias_sb["bk"][:, ec:ec + 1])

            # ---- V projection: V[s,e] tiles [128, 1024] ----
            wvT = prep_weight(wv, "wv")
            V = [v_pool.tile([P, D], BF16, tag=f"v{sc}", name=f"v{sc}")
                 for sc in range(NS)]
            for sc in range(NS):
                for eh in range(2):
                    ps = ps_sc.tile([P, QB], F32, tag="sc")
                    for dc in range(ND):
                        nc.tensor.matmul(
                            ps[:],
                            lhsT=inpT[dc][:, sc * P:(sc + 1) * P],
                            rhs=wvT[dc][:, eh * QB:(eh + 1) * QB],
                            start=(dc == 0), stop=(dc == ND - 1))
                    nc.vector.tensor_tensor(
                        out=V[sc][:, eh * QB:(eh + 1) * QB],
                        in0=ps[:], in1=bv_bcast[:, eh * QB:(eh + 1) * QB],
                        op=mybir.AluOpType.add)

            wqT = prep_weight(wq, "wq")

            # ---- attention, per q-block j ----
            for j in range(NQB):
                # QT_j[e, q] tiles [128, 512]
                QTj = [qt_pool.tile([P, QB], BF16, tag=f"qt{ec}", name=f"qt{j}_{ec}")
                       for ec in range(NE)]
                for ec in range(NE):
                    ps = ps_sc.tile([P, QB], F32, tag="sc")
                    for dc in range(ND):
                        nc.tensor.matmul(
                            ps[:],
                            lhsT=wqT[dc][:, ec * P:(ec + 1) * P],
                            rhs=inpT[dc][:, j * QB:(j + 1) * QB],
                            start=(dc == 0), stop=(dc == ND - 1))
                    nc.scalar.activation(
                        QTj[ec][:], ps[:],
                        mybir.ActivationFunctionType.Identity,
                        bias=bias_sb["bq"][:, ec:ec + 1])

                nkc = 4 * (j + 1)       # causal: k-chunks 0 .. 4j+3
                Pt = []
                for i in range(nkc):
                    ps = ps_sc.tile([P, QB], F32, tag="sc")
                    for ec in range(NE):
                        nc.tensor.matmul(
                            ps[:],
                            lhsT=KT[ec][:, i * P:(i + 1) * P],
                            rhs=QTj[ec][:],
                            start=(ec == 0), stop=(ec == NE - 1))
                    if i >= 4 * j:      # diagonal-crossing block
                        m = i - 4 * j
                        nc.vector.tensor_tensor(
                            out=ps[:], in0=ps[:],
                            in1=mask_sb[:, m * QB:(m + 1) * QB],
                            op=mybir.AluOpType.add)
                    pt = p_pool.tile([P, QB], BF16, tag="p")
                    nc.scalar.activation(pt[:], ps[:],
                                         mybir.ActivationFunctionType.Exp,
                                         scale=SCALE)
                    Pt.append(pt)

                for qs in range(4):     # q-subblocks of 128 within block j
                    qi = 4 * j + qs
                    q0 = qs * P
                    c0 = ps_ctx.tile([P, QB], F32, tag="ctx")
                    c1 = ps_ctx.tile([P, QB], F32, tag="ctx")
                    sm = ps_sum.tile([P, 1], F32, tag="sum")
                    for i in range(qi + 1):
                        lhs = Pt[i][:, q0:q0 + P]
                        first, last = (i == 0), (i == qi)
                        nc.tensor.matmul(c0[:], lhsT=lhs, rhs=V[i][:, 0:QB],
                                         start=first, stop=last)
                        nc.tensor.matmul(c1[:], lhsT=lhs, rhs=V[i][:, QB:D],
                                         start=first, stop=last)
                        nc.tensor.matmul(sm[:], lhsT=lhs, rhs=ones[:],
                                         start=first, stop=last)
                    rc = recip_pool.tile([P, 1], F32, tag="recip")
                    nc.vector.reciprocal(rc[:], sm[:])
                    ob = out_pool.tile([P, D], F32, tag="out")
                    nc.scalar.activation(ob[:, 0:QB], c0[:],
                                         mybir.ActivationFunctionType.Copy,
                                         scale=rc[:, 0:1])
                    nc.scalar.activation(ob[:, QB:D], c1[:],
                                         mybir.ActivationFunctionType.Copy,
                                         scale=rc[:, 0:1])
                    nc.sync.dma_start(out=out[qi * P:(qi + 1) * P, :], in_=ob[:])

    _split_excess_waits(nc)
    return nc


def _split_excess_waits(nc, max_waits=1):
    """This walrus build rejects instructions carrying more than one sync
    wait. Hoist excess waits onto nop instructions placed just before, on the
    same engine — semantically identical (engine blocks in program order)."""
    n_new = 0
    for f in nc.m.functions:
        for bb in f.blocks:
            insts = list(bb.instructions)
            out, changed = [], False
            for inst in insts:
                si = getattr(inst, "sync_info", None)
                if si is not None and si.on_wait and len(si.on_wait) > max_waits:
                    waits = list(si.on_wait)
                    keep, extra = waits[-max_waits:], waits[:-max_waits]
                    for i in range(0, len(extra), max_waits):
                        out.append(mybir.InstNoOp(
                            name=f"I-waitsplit-{n_new}",
                            engine=inst.engine, ins=[], outs=[],
                            sync_info=mybir.SyncInfo(
                                on_wait=extra[i:i + max_waits], on_update=[]),
                        ))
                        n_new += 1
                    si.on_wait = keep
                    changed = True
                out.append(inst)
            if changed:
                bb.instructions.clear()
                for x in out:
                    bb.instructions.append(x)
    return n_new


_NC = None


def _get_nc():
    global _NC
    if _NC is None:
        _NC = _build_nc()
    return _NC


def kernel(inp, Wq, bq, Wk, bk, Wv, bv, attn_mask):
    global LAST_RESULTS
    inp = np.ascontiguousarray(np.asarray(inp, dtype=np.float32))
    am = np.asarray(attn_mask)
    # 4 diagonal-block additive mask patterns in [k_rel, q_rel] layout
    masks4 = np.stack([
        np.where(am[0, :QB, m * P:(m + 1) * P].T, np.float32(MASKVAL),
                 np.float32(0.0))
        for m in range(4)
    ]).astype(ml_dtypes.bfloat16)

    shared = {
        "wq": np.ascontiguousarray(np.asarray(Wq, dtype=np.float32)),
        "wk": np.ascontiguousarray(np.asarray(Wk, dtype=np.float32)),
        "wv": np.ascontiguousarray(np.asarray(Wv, dtype=np.float32)),
        "bq": np.ascontiguousarray(np.asarray(bq, dtype=np.float32)),
        "bk": np.ascontiguousarray(np.asarray(bk, dtype=np.float32)),
        "bv": np.ascontiguousarray(np.asarray(bv, dtype=np.float32)),
        "masks": masks4,
    }
    in_maps = [dict(shared, inp=inp[b]) for b in range(B)]

    nc = _get_nc()
    res = run_bass_kernel_spmd(nc, in_maps, core_ids=list(range(B)),
                               trace=_TRACE)
    LAST_RESULTS = res
    return np.stack([r["out"] for r in res.results]).astype(np.float32)


# revision 6
# speedup vs baseline: 1.0999x; 1.0999x over previous
"""Causal single-head attention (B=8, S=2048, D=1024) on 8 TRN2 NeuronCores.

Sharding: data-parallel over batch — core b computes batch element b entirely.

Per-core pipeline (all matmuls bf16 with fp32 PSUM accumulation):
  1. inp [S,D] f32 -> cast bf16 -> PE-transpose (128x128, identity) -> inpT [D,S]
  2. W* [e,d] f32 -> cast bf16 -> PE-transpose -> W*T [d,e]
  3. KT[e,s] = W_kT.T @ inpT (+bk on eviction), V[s,e] = inpT.T @ WvT (+bv)
  4. per q-block j (512 wide): QT_j = WqT.T @ inpT[:, j]  (+bq)
     ST[k,q] blocks = KT.T @ QT_j, causal-masked (additive -1e30 tiles,
     only diagonal-crossing blocks; fully-masked blocks skipped entirely),
     P = exp(scale*ST) -> bf16  (no max subtraction: |scale*ST| <~ 10)
     ctx[q,e] = sum_k P[k,q].T @ V[k,e]; rowsum via ones-column matmul;
     out = ctx * (1/rowsum) on eviction.
"""

import ml_dtypes
import numpy as np

import concourse.bass as bass
import concourse.mybir as mybir
from concourse.bass_utils import run_bass_kernel_spmd
from concourse.tile import TileContext

F32 = mybir.dt.float32
BF16 = mybir.dt.bfloat16

B, S, D = 8, 2048, 1024
P = 128                # partitions
NS = S // P            # 16 s-chunks of 128
ND = D // P            # 8 d-chunks of 128
NE = D // P            # 8 e-chunks of 128
QB = 512               # q-block width (PSUM bank = 512 f32)
NQB = S // QB          # 4 q-blocks
NKC = S // P           # 16 k-chunks of 128
MASKVAL = -1.0e30
SCALE = float(np.float32(1.0) / np.sqrt(np.float32(S)))

_TRACE = False
LAST_RESULTS = None


def _build_nc():
    nc = bass.Bass()
    inp = nc.dram_tensor("inp", [S, D], F32, kind="ExternalInput")
    wq = nc.dram_tensor("wq", [D, D], F32, kind="ExternalInput")
    wk = nc.dram_tensor("wk", [D, D], F32, kind="ExternalInput")
    wv = nc.dram_tensor("wv", [D, D], F32, kind="ExternalInput")
    bq = nc.dram_tensor("bq", [D], F32, kind="ExternalInput")
    bk = nc.dram_tensor("bk", [D], F32, kind="ExternalInput")
    bv = nc.dram_tensor("bv", [D], F32, kind="ExternalInput")
    # 4 diagonal-block mask patterns, [k_rel(128), q_rel(512)], 0 or -1e30
    masks = nc.dram_tensor("masks", [4, P, QB], BF16, kind="ExternalInput")
    ident = nc.dram_tensor("ident", [P, P], BF16, kind="ExternalInput")
    out = nc.dram_tensor("out", [S, D], F32, kind="ExternalOutput")

    with TileContext(nc) as tc:
        with (
            tc.tile_pool(name="const", bufs=1) as const_pool,
            tc.tile_pool(name="stage_f", bufs=2) as stage_f,
            tc.tile_pool(name="stage_b", bufs=6) as stage_b,
            tc.tile_pool(name="wt", bufs=2) as wt_pool,
            tc.tile_pool(name="inpT", bufs=1) as inpT_pool,
            tc.tile_pool(name="kt", bufs=1) as kt_pool,
            tc.tile_pool(name="v", bufs=1) as v_pool,
            tc.tile_pool(name="qt", bufs=2) as qt_pool,
            tc.tile_pool(name="p", bufs=NKC) as p_pool,
            tc.tile_pool(name="outp", bufs=2) as out_pool,
            tc.tile_pool(name="recip", bufs=2) as recip_pool,
        ):
            with tc.tile_pool(name="ps_a", bufs=1, space="PSUM") as ps_a:
                # ---- constants ----
                ones = const_pool.tile([P, 1], BF16, tag="ones")
                nc.vector.memset(ones[:], 1.0)
                idt = const_pool.tile([P, P], BF16, tag="idt")
                nc.sync.dma_start(out=idt[:], in_=ident[:])

                bias_sb = {}
                for name, t in (("bq", bq), ("bk", bk)):
                    bt = const_pool.tile([P, NE], F32, tag=f"bias_{name}",
                                         name=f"bias_{name}")
                    nc.sync.dma_start(out=bt[:],
                                      in_=t.rearrange("(c p) -> p c", p=P))
                    bias_sb[name] = bt

                mask_sb = const_pool.tile([P, 4 * QB], BF16, tag="masks")
                for m in range(4):
                    nc.sync.dma_start(out=mask_sb[:, m * QB:(m + 1) * QB],
                                      in_=masks[m])

                # bv broadcast to all partitions via ones-outer-product matmul
                ones_row = const_pool.tile([1, P], BF16, tag="ones_row")
                nc.vector.memset(ones_row[:], 1.0)
                bv_row = stage_f.tile([1, D], F32, tag="st_f32")
                nc.sync.dma_start(out=bv_row[:], in_=bv[None, :])
                bv_row_bf = stage_b.tile([1, D], BF16, tag="st_bf")
                nc.vector.tensor_copy(bv_row_bf[:], bv_row[:])
                bv_bcast = const_pool.tile([P, D], BF16, tag="bv_bcast")
                for eh in range(2):
                    ps = ps_a.tile([P, QB], F32, tag="sc", bufs=4,
                                   name="bvb_ps")
                    nc.tensor.matmul(ps[:], lhsT=ones_row[:],
                                     rhs=bv_row_bf[:, eh * QB:(eh + 1) * QB],
                                     start=True, stop=True)
                    nc.vector.tensor_copy(bv_bcast[:, eh * QB:(eh + 1) * QB],
                                          ps[:])

                # ---- transpose helper: 4 PE transposes -> 1 PSUM tile ->
                # one DVE eviction into dst[:, g*512:(g+1)*512] ----
                def transpose_group(srcs, dst, g, nm):
                    # srcs: 4 (tile, col0) pairs -> SBUF bf16 [128,128] slices
                    tp = ps_a.tile([P, QB], BF16, tag="tr", bufs=3, name=nm)
                    for t, (src, c0) in enumerate(srcs):
                        nc.tensor.transpose(tp[:, t * P:(t + 1) * P],
                                            src[:, c0:c0 + P], idt[:])
                    nc.vector.tensor_copy(dst[:, g * QB:(g + 1) * QB], tp[:])

                # ---- input: load f32, cast bf16, PE-transpose to inpT ----
                inpT = [inpT_pool.tile([P, S], BF16, tag=f"inpT{dc}",
                                       name=f"inpT{dc}") for dc in range(ND)]
                inp_bf = []
                for si in range(NS):
                    st = stage_f.tile([P, D], F32, tag="st_f32")
                    nc.sync.dma_start(out=st[:], in_=inp[si * P:(si + 1) * P, :])
                    cb = stage_b.tile([P, D], BF16, tag="st_bf")
                    nc.vector.tensor_copy(cb[:], st[:])
                    inp_bf.append(cb)
                    if si % 4 == 3:
                        g = si // 4
                        for dc in range(ND):
                            transpose_group(
                                [(inp_bf[4 * g + t], dc * P) for t in range(4)],
                                inpT[dc], g, f"tp_inp{g}_{dc}")

                # ---- weight prep: W [e,d] -> wT tiles [128d, 1024e] bf16 ----
                def prep_weight(w, wname):
                    wT = [wt_pool.tile([P, D], BF16, tag=f"wt{dc}",
                                       name=f"{wname}T{dc}")
                          for dc in range(ND)]
                    w_bf = []
                    for ec in range(NE):
                        st = stage_f.tile([P, D], F32, tag="st_f32")
                        nc.sync.dma_start(out=st[:],
                                          in_=w[ec * P:(ec + 1) * P, :])
                        cb = stage_b.tile([P, D], BF16, tag="st_bf")
                        nc.scalar.activation(cb[:], st[:],
                                             mybir.ActivationFunctionType.Copy)
                        w_bf.append(cb)
                        if ec % 4 == 3:
                            g = ec // 4
                            for dc in range(ND):
                                transpose_group(
                                    [(w_bf[4 * g + t], dc * P)
                                     for t in range(4)],
                                    wT[dc], g, f"tp_{wname}{g}_{dc}")
                    return wT

                # ---- K projection: KT[e,s] tiles [128, 2048] ----
                wkT = prep_weight(wk, "wk")
                KT = [kt_pool.tile([P, S], BF16, tag=f"kt{ec}", name=f"kt{ec}")
                      for ec in range(NE)]
                for ec in range(NE):
                    for sb in range(NQB):
                        ps = ps_a.tile([P, QB], F32, tag="sc", bufs=4,
                                       name="kt_ps")
                        for dc in range(ND):
                            nc.tensor.matmul(
                                ps[:],
                                lhsT=wkT[dc][:, ec * P:(ec + 1) * P],
                                rhs=inpT[dc][:, sb * QB:(sb + 1) * QB],
                                start=(dc == 0), stop=(dc == ND - 1))
                        nc.scalar.activation(
                            KT[ec][:, sb * QB:(sb + 1) * QB], ps[:],
                            mybir.ActivationFunctionType.Identity,
                            bias=bias_sb["bk"][:, ec:ec + 1])

                # ---- V projection: V[s,e] tiles [128, 1024] ----
                wvT = prep_weight(wv, "wv")
                V = [v_pool.tile([P, D], BF16, tag=f"v{sc}", name=f"v{sc}")
                     for sc in range(NS)]
                for sc in range(NS):
                    for eh in range(2):
                        ps = ps_a.tile([P, QB], F32, tag="sc", bufs=4,
                                       name="v_ps")
                        for dc in range(ND):
                            nc.tensor.matmul(
                                ps[:],
                                lhsT=inpT[dc][:, sc * P:(sc + 1) * P],
                                rhs=wvT[dc][:, eh * QB:(eh + 1) * QB],
                                start=(dc == 0), stop=(dc == ND - 1))
                        nc.vector.tensor_tensor(
                            out=V[sc][:, eh * QB:(eh + 1) * QB],
                            in0=ps[:], in1=bv_bcast[:, eh * QB:(eh + 1) * QB],
                            op=mybir.AluOpType.add)

                wqT = prep_weight(wq, "wq")

            # ---- attention, per q-block j ----
            with tc.tile_pool(name="ps_b", bufs=1, space="PSUM") as ps_b:
                for j in range(NQB):
                    # QT_j[e, q] tiles [128, 512]
                    QTj = [qt_pool.tile([P, QB], BF16, tag=f"qt{ec}",
                                        name=f"qt{j}_{ec}")
                           for ec in range(NE)]
                    for ec in range(NE):
                        ps = ps_b.tile([P, QB], F32, tag="sc", bufs=4,
                                       name="qt_ps")
                        for dc in range(ND):
                            nc.tensor.matmul(
                                ps[:],
                                lhsT=wqT[dc][:, ec * P:(ec + 1) * P],
                                rhs=inpT[dc][:, j * QB:(j + 1) * QB],
                                start=(dc == 0), stop=(dc == ND - 1))
                        nc.scalar.activation(
                            QTj[ec][:], ps[:],
                            mybir.ActivationFunctionType.Identity,
                            bias=bias_sb["bq"][:, ec:ec + 1])

                    nkc = 4 * (j + 1)       # causal: k-chunks 0 .. 4j+3
                    Pt = []
                    for i in range(nkc):
                        ps = ps_b.tile([P, QB], F32, tag="sc", bufs=4,
                                       name="st_ps")
                        for ec in range(NE):
                            nc.tensor.matmul(
                                ps[:],
                                lhsT=KT[ec][:, i * P:(i + 1) * P],
                                rhs=QTj[ec][:],
                                start=(ec == 0), stop=(ec == NE - 1))
                        if i >= 4 * j:      # diagonal-crossing block
                            m = i - 4 * j
                            nc.vector.tensor_tensor(
                                out=ps[:], in0=ps[:],
                                in1=mask_sb[:, m * QB:(m + 1) * QB],
                                op=mybir.AluOpType.add)
                        pt = p_pool.tile([P, QB], BF16, tag="p",
                                         name=f"p{j}_{i}")
                        nc.scalar.activation(pt[:], ps[:],
                                             mybir.ActivationFunctionType.Exp,
                                             scale=SCALE)
                        Pt.append(pt)

                    for qs in range(4):     # q-subblocks of 128 within j
                        qi = 4 * j + qs
                        q0 = qs * P
                        c0 = ps_b.tile([P, QB], F32, tag="ctx", bufs=3,
                                       name="c0_ps")
                        c1 = ps_b.tile([P, QB], F32, tag="ctx", bufs=3,
                                       name="c1_ps")
                        sm = ps_b.tile([P, 1], F32, tag="sum", bufs=1,
                                       name="sum_ps")
                        for i in range(qi + 1):
                            lhs = Pt[i][:, q0:q0 + P]
                            first, last = (i == 0), (i == qi)
                            nc.tensor.matmul(c0[:], lhsT=lhs,
                                             rhs=V[i][:, 0:QB],
                                             start=first, stop=last)
                            nc.tensor.matmul(c1[:], lhsT=lhs,
                                             rhs=V[i][:, QB:D],
                                             start=first, stop=last)
                            nc.tensor.matmul(sm[:], lhsT=lhs, rhs=ones[:],
                                             start=first, stop=last)
                        rc = recip_pool.tile([P, 1], F32, tag="recip",
                                             name="recip")
                        nc.vector.reciprocal(rc[:], sm[:])
                        ob = out_pool.tile([P, D], F32, tag="out", name="ob")
                        nc.vector.tensor_scalar_mul(ob[:, 0:QB], c0[:],
                                                    rc[:, 0:1])
                        nc.vector.tensor_scalar_mul(ob[:, QB:D], c1[:],
                                                    rc[:, 0:1])
                        nc.sync.dma_start(out=out[qi * P:(qi + 1) * P, :],
                                          in_=ob[:])

    _split_excess_waits(nc)
    return nc


def _split_excess_waits(nc, max_waits=1):
    """This walrus build rejects instructions carrying more than one sync
    wait. Hoist excess waits onto nop instructions placed just before, on the
    same engine — semantically identical (engine blocks in program order)."""
    n_new = 0
    for f in nc.m.functions:
        for bb in f.blocks:
            insts = list(bb.instructions)
            out, changed = [], False
            for inst in insts:
                si = getattr(inst, "sync_info", None)
                if si is not None and si.on_wait and len(si.on_wait) > max_waits:
                    waits = list(si.on_wait)
                    keep, extra = waits[-max_waits:], waits[:-max_waits]
                    for i in range(0, len(extra), max_waits):
                        out.append(mybir.InstNoOp(
                            name=f"I-waitsplit-{n_new}",
                            engine=inst.engine, ins=[], outs=[],
                            sync_info=mybir.SyncInfo(
                                on_wait=extra[i:i + max_waits], on_update=[]),
                        ))
                        n_new += 1
                    si.on_wait = keep
                    changed = True
                out.append(inst)
            if changed:
                bb.instructions.clear()
                for x in out:
                    bb.instructions.append(x)
    return n_new


_NC = None


def _get_nc():
    global _NC
    if _NC is None:
        _NC = _build_nc()
    return _NC


def kernel(inp, Wq, bq, Wk, bk, Wv, bv, attn_mask):
    global LAST_RESULTS
    inp = np.ascontiguousarray(np.asarray(inp, dtype=np.float32))
    am = np.asarray(attn_mask)
    # 4 diagonal-block additive mask patterns in [k_rel, q_rel] layout
    masks4 = np.stack([
        np.where(am[0, :QB, m * P:(m + 1) * P].T, np.float32(MASKVAL),
                 np.float32(0.0))
        for m in range(4)
    ]).astype(ml_dtypes.bfloat16)

    shared = {
        "wq": np.ascontiguousarray(np.asarray(Wq, dtype=np.float32)),
        "wk": np.ascontiguousarray(np.asarray(Wk, dtype=np.float32)),
        "wv": np.ascontiguousarray(np.asarray(Wv, dtype=np.float32)),
        "bq": np.ascontiguousarray(np.asarray(bq, dtype=np.float32)),
        "bk": np.ascontiguousarray(np.asarray(bk, dtype=np.float32)),
        "bv": np.ascontiguousarray(np.asarray(bv, dtype=np.float32)),
        "masks": masks4,
        "ident": np.eye(P, dtype=ml_dtypes.bfloat16),
    }
    in_maps = [dict(shared, inp=inp[b]) for b in range(B)]

    nc = _get_nc()
    res = run_bass_kernel_spmd(nc, in_maps, core_ids=list(range(B)),
                               trace=_TRACE)
    LAST_RESULTS = res
    return np.stack([r["out"] for r in res.results]).astype(np.float32)
